# revision 2
# baseline (speedup 1.0000x reference)
"""Trainium2 Bass kernel for the batched Kalman filter problem.

Shapes (hardcoded per the problem spec): G=1024 groups, T=200 timesteps,
S=16 state dims, M=4 measurement dims.  8 NeuronCores, data-parallel over
G (128 groups per core).

Math: every group shares F/Q/H/R and the same init_cov, so the covariance
recursion (P_t, innovation cov, Kalman gain) is group-independent: the
covs / meas_covs outputs are a single [T,S,S] / [T,M,M] sequence broadcast
over G, and the group-dependent part collapses to a time-varying affine
recurrence on the mean:

    mean[t+1] = A_t mean[t] + B_t y_t,   A_t = F (I - K_t H),  B_t = F K_t.

The tiny [16,16] covariance recursion and the block weight matrices derived
from it are computed on host in float64 (they depend only on the small
parameter matrices, not on the data).  The device kernel does all the
data-proportional work: for each block of BLK=25 timesteps it evaluates

    out[g, (r,s)]  = mean[t0+r][s]      (400 cols)
    out[g, (r,m)]  = H mean[t0+r][m]    (100 cols)

as one PE matmul per block with the data Z_j = [mean[t0]; y-block] [116,128]
stationary and the weight matrix W_j [116,500] moving, plus a small serial
boundary-chain matmul per block to produce mean[t0+BLK] for the next block.
"""

import os

import numpy as np

G, T, S, M = 1024, 200, 16, 4
NCORES = 8
GC = G // NCORES          # groups per core = 128
BLK = 25                  # timesteps per block
NBLK = (T - 1 + BLK - 1) // BLK   # 8
K_IN = S + M * BLK        # 116 contraction rows: 16 mean + 100 obs
N_MEAN = S * BLK          # 400 mean output columns
N_OUT = N_MEAN + M * BLK  # 500 total output columns
COV_ROWS, COV_COLS = 128, (T * S * S) // 128      # [128, 400]
MCOV_ROWS, MCOV_COLS = 128, (T * M * M) // 128    # [128, 25]

_CACHE = {}


# ----------------------------------------------------------------------------
# Host-side math (parameter-only, data-independent)
# ----------------------------------------------------------------------------

def _cov_sequence(F, Q, H, R, P0):
    """P_t for t=0..T-1, meas_cov_t = H P_t H^T + R, and the mean-recurrence
    coefficients A_t, B_t for t=0..T-2.  float64 internally."""
    F = F.astype(np.float64)
    Q = Q.astype(np.float64)
    H = H.astype(np.float64)
    R = R.astype(np.float64)
    P = P0.astype(np.float64)
    covs = np.empty((T, S, S), np.float64)
    meas_covs = np.empty((T, M, M), np.float64)
    A = np.empty((T - 1, S, S), np.float64)
    B = np.empty((T - 1, S, M), np.float64)
    I = np.eye(S)
    for t in range(T):
        covs[t] = P
        meas_covs[t] = H @ P @ H.T + R
        if t == T - 1:
            break
        HP = H @ P
        Smat = HP @ H.T + R
        K = np.linalg.solve(Smat, HP).T  # [S,M]
        A[t] = F @ (I - K @ H)
        B[t] = F @ K
        P = F @ (P - K @ HP) @ F.T + Q
        P = 0.5 * (P + P.T)
    return covs, meas_covs, A, B


def _block_weights(A, B, H):
    """Per-block weight matrices W[j] [K_IN, N_OUT] such that with
    Z rows p<16 = mean[t0][p], p=16+4i+m = y[t0+i][m]:
      out[g, (r-1)*16+s]        = mean[t0+r][s]
      out[g, N_MEAN+(r-1)*4+mm] = (H mean[t0+r])[mm]        for r=1..BLK."""
    H = H.astype(np.float64)
    W = np.zeros((NBLK, K_IN, N_OUT), np.float64)
    for j in range(NBLK):
        t0 = BLK * j
        rmax = min(BLK, (T - 1) - t0)
        C = np.eye(S)
        D = np.zeros((BLK, S, M))
        for r in range(1, rmax + 1):
            t = t0 + r - 1
            C = A[t] @ C
            for i in range(r - 1):
                D[i] = A[t] @ D[i]
            D[r - 1] = B[t]
            o = (r - 1) * S
            om = N_MEAN + (r - 1) * M
            W[j, 0:S, o:o + S] = C.T
            W[j, 0:S, om:om + M] = (H @ C).T
            for i in range(r):
                p = S + M * i
                W[j, p:p + M, o:o + S] = D[i].T
                W[j, p:p + M, om:om + M] = (H @ D[i]).T
    return W.astype(np.float32)


# ----------------------------------------------------------------------------
# Device kernel
# ----------------------------------------------------------------------------

def _build_module():
    import concourse.bacc as bacc
    import concourse.tile as tile
    from concourse import mybir

    nc = bacc.Bacc(
        "TRN2",
        target_bir_lowering=False,
        debug=False,
        enable_asserts=False,
        num_devices=NCORES,
    )
    f32 = mybir.dt.float32
    y_in = nc.dram_tensor("y", [NBLK, M * BLK, GC], f32, kind="ExternalInput").ap()
    zinit = nc.dram_tensor("zinit", [S, GC], f32, kind="ExternalInput").ap()
    w_in = nc.dram_tensor("w", [NBLK, K_IN, N_OUT], f32, kind="ExternalInput").ap()
    covs_in = nc.dram_tensor("covs_in", [COV_ROWS, COV_COLS], f32, kind="ExternalInput").ap()
    mcovs_in = nc.dram_tensor("mcovs_in", [MCOV_ROWS, MCOV_COLS], f32, kind="ExternalInput").ap()
    out_blocks = nc.dram_tensor("out_blocks", [NBLK, GC, N_OUT], f32, kind="ExternalOutput").ap()
    covs_out = nc.dram_tensor("covs_out", [COV_ROWS, COV_COLS], f32, kind="ExternalOutput").ap()
    mcovs_out = nc.dram_tensor("mcovs_out", [MCOV_ROWS, MCOV_COLS], f32, kind="ExternalOutput").ap()

    with tile.TileContext(nc) as tc:
        with tc.tile_pool(name="persist", bufs=1) as persist, \
             tc.tile_pool(name="obuf", bufs=3) as obuf, \
             tc.tile_pool(name="pp", bufs=4, space="PSUM") as pp, \
             tc.tile_pool(name="sp", bufs=2, space="PSUM") as sp:

            # covs / meas_covs passthrough (host-computed, group-independent)
            cov_sb = persist.tile([COV_ROWS, COV_COLS], f32, name="cov_sb")
            nc.sync.dma_start(out=cov_sb, in_=covs_in)
            nc.sync.dma_start(out=covs_out, in_=cov_sb)
            mcov_sb = persist.tile([MCOV_ROWS, MCOV_COLS], f32, name="mcov_sb")
            nc.sync.dma_start(out=mcov_sb, in_=mcovs_in)
            nc.sync.dma_start(out=mcovs_out, in_=mcov_sb)

            Z, W = [], []
            for j in range(NBLK):
                zt = persist.tile([K_IN, GC], f32, name=f"z{j}", tag=f"z{j}")
                nc.sync.dma_start(out=zt[S:K_IN, :], in_=y_in[j])
                if j == 0:
                    nc.sync.dma_start(out=zt[0:S, :], in_=zinit)
                wt = persist.tile([K_IN, N_OUT], f32, name=f"w{j}", tag=f"w{j}")
                nc.sync.dma_start(out=wt, in_=w_in[j])
                Z.append(zt)
                W.append(wt)

            # serial boundary chain: mean[t0+BLK] -> rows 0:16 of next Z
            for j in range(NBLK - 1):
                st = sp.tile([S, GC], f32, name=f"st{j}", tag="st")
                nc.tensor.matmul(
                    st, lhsT=W[j][:, N_MEAN - S:N_MEAN], rhs=Z[j],
                    start=True, stop=True,
                )
                nc.scalar.copy(out=Z[j + 1][0:S, :], in_=st)

            # per-block dense evaluation: out[g, :] for 25 timesteps at once
            for j in range(NBLK):
                ot = pp.tile([GC, N_OUT], f32, name=f"ot{j}", tag="ot")
                nc.tensor.matmul(ot, lhsT=Z[j], rhs=W[j], start=True, stop=True)
                ob = obuf.tile([GC, N_OUT], f32, name=f"ob{j}", tag="ob")
                nc.vector.tensor_copy(out=ob, in_=ot)
                nc.sync.dma_start(out=out_blocks[j], in_=ob)

    nc.compile()
    return nc


def _get_module():
    if "nc" not in _CACHE:
        _CACHE["nc"] = _build_module()
    return _CACHE["nc"]


# ----------------------------------------------------------------------------
# Entry point
# ----------------------------------------------------------------------------

def kernel(input, F, Q, H, R, init_mean, init_cov):
    from concourse.bass_utils import run_bass_kernel_spmd

    input = np.ascontiguousarray(np.asarray(input, np.float32))
    F = np.asarray(F, np.float32)
    Q = np.asarray(Q, np.float32)
    H = np.asarray(H, np.float32)
    R = np.asarray(R, np.float32)
    init_mean = np.asarray(init_mean, np.float32)
    init_cov = np.asarray(init_cov, np.float32)

    # The fast path relies on init_cov being identical across groups (true for
    # this problem: broadcast identity).  Guard it; fall back to a plain
    # host filter if violated so correctness never depends on the assumption.
    if np.ptp(init_cov, axis=0).max() != 0.0:
        return _host_fallback(input, F, Q, H, R, init_mean, init_cov)

    covs64, mcovs64, A, B = _cov_sequence(F, Q, H, R, init_cov[0])
    Wfull = _block_weights(A, B, H)                       # [NBLK, K_IN, N_OUT]
    covs32 = np.ascontiguousarray(covs64.astype(np.float32))
    mcovs32 = np.ascontiguousarray(mcovs64.astype(np.float32))

    # y stream: y_blocks[j, 4*i+m, g] = input[g, 25j+i, m]; obs index 25j+i
    # runs to T-2=198 (the scan consumes input[:, :-1, :]); pad the tail.
    ypad = np.zeros((NBLK * BLK, M, G), np.float32)
    ypad[:T - 1] = input[:, :T - 1, :].transpose(1, 2, 0)
    y_blocks = np.ascontiguousarray(
        ypad.reshape(NBLK, BLK * M, G))                   # [8, 100, 1024]

    covs_flat = covs32.reshape(COV_ROWS, COV_COLS)
    mcovs_flat = mcovs32.reshape(MCOV_ROWS, MCOV_COLS)

    in_maps = []
    for c in range(NCORES):
        gs = slice(c * GC, (c + 1) * GC)
        in_maps.append({
            "y": np.ascontiguousarray(y_blocks[:, :, gs]),
            "zinit": np.ascontiguousarray(init_mean[gs].T),
            "w": Wfull,
            "covs_in": covs_flat,
            "mcovs_in": mcovs_flat,
        })

    nc = _get_module()
    res = run_bass_kernel_spmd(
        nc, in_maps, core_ids=list(range(NCORES)),
        trace=bool(os.environ.get("KF_TRACE")),
    )
    _CACHE["last_results"] = res

    means = np.empty((G, T, S), np.float32)
    meas_means = np.empty((G, T, M), np.float32)
    means[:, 0, :] = init_mean
    meas_means[:, 0, :] = init_mean @ H.T
    for c in range(NCORES):
        gs = slice(c * GC, (c + 1) * GC)
        ob = res.results[c]["out_blocks"]                 # [NBLK, GC, N_OUT]
        mean_part = ob[:, :, :N_MEAN].reshape(NBLK, GC, BLK, S)
        meas_part = ob[:, :, N_MEAN:].reshape(NBLK, GC, BLK, M)
        for j in range(NBLK):
            t0 = BLK * j
            rmax = min(BLK, (T - 1) - t0)
            means[gs, t0 + 1:t0 + 1 + rmax, :] = mean_part[j][:, :rmax, :]
            meas_means[gs, t0 + 1:t0 + 1 + rmax, :] = meas_part[j][:, :rmax, :]

    covs_dev = res.results[0]["covs_out"].reshape(T, S, S)
    mcovs_dev = res.results[0]["mcovs_out"].reshape(T, M, M)
    covs = np.broadcast_to(covs_dev[None], (G, T, S, S))
    meas_covs = np.broadcast_to(mcovs_dev[None], (G, T, M, M))
    return means, covs, meas_means, meas_covs


def _host_fallback(input, F, Q, H, R, init_mean, init_cov):
    """Reference-equivalent numpy filter (defensive path, not expected to run)."""
    Gn, Tn, _ = input.shape
    mean = init_mean.astype(np.float64)
    cov = init_cov.astype(np.float64)
    F64, Q64, H64, R64 = (x.astype(np.float64) for x in (F, Q, H, R))
    means = np.empty((Gn, Tn, S), np.float32)
    covs = np.empty((Gn, Tn, S, S), np.float32)
    means[:, 0] = mean
    covs[:, 0] = cov
    for t in range(Tn - 1):
        obs = input[:, t, :].astype(np.float64)
        HP = np.einsum('ms,gsk->gmk', H64, cov)
        Smat = HP @ H64.T + R64
        K = np.swapaxes(np.linalg.solve(Smat, HP), 1, 2)
        resid = obs - mean @ H64.T
        mean_u = mean + np.einsum('gsm,gm->gs', K, resid)
        cov_u = cov - K @ HP
        mean = mean_u @ F64.T
        cov = np.einsum('ij,gjk,lk->gil', F64, cov_u, F64) + Q64
        means[:, t + 1] = mean
        covs[:, t + 1] = cov
    meas_means = np.einsum('gts,ms->gtm', means, H).astype(np.float32)
    HPc = np.einsum('ms,gtsk->gtmk', H, covs)
    meas_covs = (np.einsum('gtmk,nk->gtmn', HPc, H) + R).astype(np.float32)
    return means, covs, meas_means, meas_covs
